# revision 3
# baseline (speedup 1.0000x reference)
"""VQ codebook kernel for Trainium2 (8 NeuronCores, batch-per-core data parallel).

Reference computation (per token x, codebook cb (K,d)):
    dist(x, k)    = ||x||^2 + ||cb_k||^2 - 2 x.cb_k      (fp32)
    idx           = argmin_k dist
    prob          = softmax(-dist)
    z_q           = cb[idx]  (+ straight-through: z + (z_q - z))
    q_loss        = 1.25 * mean((z_q - z)^2)

Precision strategy: the reference's argmin is decided by fp32 rounding of
dist at magnitude ~512 (ulp 6.1e-5) while top-2 gaps go down to 1e-8, so we
mirror the reference's fp32 arithmetic exactly:
    t1 = fp32(A + B)  with A=||x||^2 (host, numpy pairwise sum = XLA CPU),
                           B=||cb||^2 row
    C2 = 2*x.cb accumulated in PSUM via a 3-pass bf16 hi/lo split (the
         factor 2 is folded into the codebook operand; exact under fp32)
    nd = fp32(C2 - t1) = -fp32(t1 - C2) = -dist  elementwise
    argmax(nd) == argmin(dist), first-occurrence tie-break on both sides.
Validated in numpy: 0/32768 argmin mismatches vs the jax-CPU reference.

Per core (batch b): z[b] is natively (d=512, tok=4096) which is exactly the
transposed stationary operand the TensorEngine wants.
"""
import sys

sys.path.insert(0, "/opt/trn_rl_repo")

import time

import numpy as np
import ml_dtypes

import concourse.bacc as bacc
import concourse.bass as bass
import concourse.mybir as mybir
import concourse.tile as tile
from concourse.bass_utils import run_bass_kernel_spmd
from concourse.masks import make_identity

B, D, H, W, K = 8, 512, 64, 64, 1024
N = H * W          # tokens per core
NT = N // 128      # 32 token tiles per core
DC = D // 128      # 4 contraction chunks
BETA = 0.25

f32 = mybir.dt.float32
bf16 = mybir.dt.bfloat16
u32 = mybir.dt.uint32

_CACHE = {}


def _build():
    nc = bacc.Bacc()

    z = nc.dram_tensor("z", [D, N], f32, kind="ExternalInput")
    zp = nc.dram_tensor("zp", [D, N], f32, kind="ExternalInput")
    cbh2 = nc.dram_tensor("cbh2", [D, K], bf16, kind="ExternalInput")
    cbl2 = nc.dram_tensor("cbl2", [D, K], bf16, kind="ExternalInput")
    cbtab = nc.dram_tensor("cbtab", [K, D], f32, kind="ExternalInput")
    arow = nc.dram_tensor("arow", [128, NT], f32, kind="ExternalInput")
    brow = nc.dram_tensor("brow", [1, K], f32, kind="ExternalInput")
    nbrow = nc.dram_tensor("nbrow", [1, K], f32, kind="ExternalInput")

    prob = nc.dram_tensor("prob", [N, K], f32, kind="ExternalOutput")
    pprob = nc.dram_tensor("pprob", [N, K], f32, kind="ExternalOutput")
    zq = nc.dram_tensor("zq", [D, N], f32, kind="ExternalOutput")
    idx = nc.dram_tensor("idx", [128, NT], u32, kind="ExternalOutput")
    lossp = nc.dram_tensor("lossp", [128, 1], f32, kind="ExternalOutput")

    z_r = z.rearrange("(c p) n -> p c n", p=128)
    zp_r = zp.rearrange("(c p) n -> p c n", p=128)
    zq_r = zq.rearrange("(c p) n -> p c n", p=128)
    Exp = mybir.ActivationFunctionType.Exp
    Square = mybir.ActivationFunctionType.Square

    with tile.TileContext(nc) as tc:
        with (
            tc.tile_pool(name="const", bufs=1) as cpool,
            tc.tile_pool(name="sbuf", bufs=2) as sp,
            tc.tile_pool(name="psum", bufs=1, space="PSUM") as pp,
        ):
            # ---------------- constants ----------------
            ch = cpool.tile([128, DC, K], bf16, tag="cbh2")
            nc.sync.dma_start(out=ch[:], in_=cbh2.rearrange("(c p) k -> p c k", p=128))
            cl = cpool.tile([128, DC, K], bf16, tag="cbl2")
            nc.sync.dma_start(out=cl[:], in_=cbl2.rearrange("(c p) k -> p c k", p=128))
            at = cpool.tile([128, NT], f32, tag="arow")
            nc.sync.dma_start(out=at[:], in_=arow[:])
            bt = cpool.tile([1, K], f32, tag="brow")
            nc.sync.dma_start(out=bt[:], in_=brow[:])
            nbt = cpool.tile([1, K], f32, tag="nbrow")
            nc.sync.dma_start(out=nbt[:], in_=nbrow[:])
            ones = cpool.tile([1, 128], f32, tag="ones")
            nc.vector.memset(ones[:], 1.0)
            ident = cpool.tile([128, 128], f32, tag="ident")
            make_identity(nc, ident[:])
            lossacc = cpool.tile([128, 1], f32, tag="lossacc")
            nc.vector.memset(lossacc[:], 0.0)
            idxacc = cpool.tile([128, NT], u32, tag="idxacc")

            # replicate B across partitions: ones(1,128).T @ B(1,K) = B rows
            bbc = cpool.tile([128, K], f32, tag="bbc")
            for ki in range(2):
                ks = slice(ki * 512, (ki + 1) * 512)
                bb_ps = pp.tile([128, 512], f32, tag="tr", bufs=4)
                nc.tensor.matmul(
                    bb_ps[:], ones[:], bt[:, ks], start=True, stop=True
                )
                nc.vector.tensor_copy(bbc[:, ks], bb_ps[:])

            # ---------------- z pass ----------------
            for t in range(NT):
                ts = slice(t * 128, (t + 1) * 128)
                zt = sp.tile([128, DC, 128], f32, tag="zt")
                nc.sync.dma_start(out=zt[:], in_=z_r[:, :, ts])
                zh = sp.tile([128, DC, 128], bf16, tag="zh")
                nc.scalar.copy(zh[:], zt[:])
                zl = sp.tile([128, DC, 128], bf16, tag="zl")
                nc.vector.tensor_sub(zl[:], zt[:], zh[:])

                nd_ps = pp.tile([128, K], f32, tag="nd", bufs=2)
                for ki in range(2):
                    ks = slice(ki * 512, (ki + 1) * 512)
                    first = True
                    for zi, ci in ((zh, ch), (zl, ch), (zh, cl)):
                        for c in range(DC):
                            nc.tensor.matmul(
                                nd_ps[:, ks],
                                zi[:, c, :],
                                ci[:, c, ks],
                                start=first,
                                stop=(zi is zh and ci is cl and c == DC - 1),
                            )
                            first = False

                # t1 = fp32(A + B), nd = fp32(C2 - t1)
                t1 = sp.tile([128, K], f32, tag="t1")
                nc.vector.tensor_scalar(
                    t1[:],
                    bbc[:],
                    at[:, t : t + 1],
                    None,
                    op0=mybir.AluOpType.add,
                )
                ndt = sp.tile([128, K], f32, tag="ndt")
                nc.vector.tensor_sub(ndt[:], nd_ps[:], t1[:])

                # argmax + softmax
                mx = sp.tile([128, 8], f32, tag="mx")
                nc.vector.max(out=mx[:], in_=ndt[:])
                mi = sp.tile([128, 8], u32, tag="mi")
                nc.vector.max_index(out=mi[:], in_max=mx[:], in_values=ndt[:])
                nc.vector.tensor_copy(idxacc[:, t : t + 1], mi[:, 0:1])
                negmax = sp.tile([128, 1], f32, tag="negmax")
                nc.vector.tensor_scalar_mul(negmax[:], mx[:, 0:1], -1.0)

                et = sp.tile([128, K], f32, tag="et")
                sume = sp.tile([128, 1], f32, tag="sume")
                nc.scalar.activation(
                    et[:], ndt[:], Exp, bias=negmax[:, 0:1], accum_out=sume[:]
                )
                rcp = sp.tile([128, 1], f32, tag="rcp")
                nc.vector.reciprocal(rcp[:], sume[:])
                pt = sp.tile([128, K], f32, tag="pt")
                nc.vector.tensor_scalar_mul(pt[:], et[:], rcp[:, 0:1])
                nc.sync.dma_start(out=prob[ts, :], in_=pt[:])

                # gather z_q rows, transpose to (d, tok)
                gt = sp.tile([128, D], f32, tag="gt")
                nc.gpsimd.indirect_dma_start(
                    out=gt[:],
                    out_offset=None,
                    in_=cbtab[:],
                    in_offset=bass.IndirectOffsetOnAxis(ap=mi[:, 0:1], axis=0),
                )
                zqt = sp.tile([128, DC, 128], f32, tag="zqt")
                diff = sp.tile([128, DC, 128], f32, tag="diff")
                for c in range(DC):
                    tr_ps = pp.tile([128, 128], f32, tag="tr", bufs=4)
                    nc.tensor.transpose(
                        out=tr_ps[:], in_=gt[:, c * 128 : (c + 1) * 128],
                        identity=ident[:],
                    )
                    # diff = z_q - z ; zqt = z + diff  (mirrors the reference STE)
                    nc.vector.tensor_sub(diff[:, c, :], tr_ps[:], zt[:, c, :])
                nc.gpsimd.tensor_add(zqt[:], diff[:], zt[:])
                nc.sync.dma_start(out=zq_r[:, :, ts], in_=zqt[:])

                sq = sp.tile([128, DC, 128], f32, tag="sq")
                lp = sp.tile([128, 1], f32, tag="lp")
                nc.scalar.activation(sq[:], diff[:], Square, accum_out=lp[:])
                nc.vector.tensor_add(lossacc[:], lossacc[:], lp[:])

            # ---------------- z_pos pass ----------------
            for t in range(NT):
                ts = slice(t * 128, (t + 1) * 128)
                zpt = sp.tile([128, DC, 128], f32, tag="zpt")
                nc.sync.dma_start(out=zpt[:], in_=zp_r[:, :, ts])
                zph = sp.tile([128, DC, 128], bf16, tag="zph")
                nc.scalar.copy(zph[:], zpt[:])

                ndp_ps = pp.tile([128, K], f32, tag="nd", bufs=2)
                for ki in range(2):
                    ks = slice(ki * 512, (ki + 1) * 512)
                    for c in range(DC):
                        nc.tensor.matmul(
                            ndp_ps[:, ks],
                            zph[:, c, :],
                            ch[:, c, ks],
                            start=(c == 0),
                            stop=False,
                        )
                    nc.tensor.matmul(
                        ndp_ps[:, ks], ones[:], nbt[:, ks], start=False, stop=True
                    )

                ep = sp.tile([128, K], f32, tag="et")
                sump = sp.tile([128, 1], f32, tag="sume")
                nc.scalar.activation(ep[:], ndp_ps[:], Exp, accum_out=sump[:])
                rcpp = sp.tile([128, 1], f32, tag="rcp")
                nc.vector.reciprocal(rcpp[:], sump[:])
                ppt = sp.tile([128, K], f32, tag="pt")
                nc.vector.tensor_scalar_mul(ppt[:], ep[:], rcpp[:, 0:1])
                nc.sync.dma_start(out=pprob[ts, :], in_=ppt[:])

            nc.sync.dma_start(out=idx[:], in_=idxacc[:])
            nc.sync.dma_start(out=lossp[:], in_=lossacc[:])

    nc.finalize()
    return nc


def _get_nc():
    if "nc" not in _CACHE:
        _CACHE["nc"] = _build()
    return _CACHE["nc"]


def kernel(z, z_pos, codebook):
    z = np.ascontiguousarray(np.asarray(z, dtype=np.float32))
    z_pos = np.ascontiguousarray(np.asarray(z_pos, dtype=np.float32))
    cb = np.ascontiguousarray(np.asarray(codebook, dtype=np.float32))

    # host prep: codebook derivatives (tiny) + per-token ||x||^2 (mirrors
    # the reference's fp32 row-sum bit-for-bit via numpy pairwise sum)
    cbT2 = np.ascontiguousarray(cb.T) * np.float32(2.0)        # (D, K)
    cbh2 = cbT2.astype(ml_dtypes.bfloat16)
    cbl2 = (cbT2 - cbh2.astype(np.float32)).astype(ml_dtypes.bfloat16)
    brow = (cb * cb).sum(axis=1).astype(np.float32)[None, :]   # (1, K)
    nbrow = -brow

    zf = z.transpose(0, 2, 3, 1).reshape(-1, D)                # (B*N, D)
    A = np.einsum("nd,nd->n", zf, zf, optimize=False)          # fallback below
    # einsum may use a different accumulation than jnp.sum; use the exact
    # elementwise-square + sum that matched the reference in validation:
    A = (zf * zf).sum(axis=1)                                  # fp32 pairwise

    in_maps = []
    for b in range(B):
        arow_b = np.ascontiguousarray(
            A[b * N : (b + 1) * N].reshape(NT, 128).T
        )  # (128, NT), arow[p, t] = A[t*128 + p]
        in_maps.append(
            {
                "z": z[b].reshape(D, N),
                "zp": z_pos[b].reshape(D, N),
                "cbh2": cbh2,
                "cbl2": cbl2,
                "cbtab": cb,
                "arow": arow_b,
                "brow": brow,
                "nbrow": nbrow,
            }
        )

    nc = _get_nc()
    t0 = time.perf_counter()
    res = run_bass_kernel_spmd(nc, in_maps, core_ids=list(range(B)))
    t1 = time.perf_counter()
    _CACHE["last_exec_wall_s"] = t1 - t0

    z_q = np.empty((B, D, H, W), dtype=np.float32)
    dist_prob = np.empty((B, H, W, K), dtype=np.float32)
    pos_prob = np.empty((B, H, W, K), dtype=np.float32)
    indices = np.empty((B, H, W), dtype=np.int32)
    loss_sum = 0.0
    for b in range(B):
        r = res.results[b]
        z_q[b] = r["zq"].reshape(D, H, W)
        dist_prob[b] = r["prob"].reshape(H, W, K)
        pos_prob[b] = r["pprob"].reshape(H, W, K)
        indices[b] = r["idx"].T.reshape(H, W).astype(np.int32)
        loss_sum += r["lossp"].astype(np.float64).sum()

    q_loss = np.float32((1.0 + BETA) * loss_sum / (B * N * D))
    return z_q, q_loss, dist_prob, indices, pos_prob


# revision 9
# speedup vs baseline: 1.9539x; 1.9539x over previous
"""VQ codebook kernel for Trainium2 (8 NeuronCores, batch-per-core data parallel).

Reference computation (per token x, codebook cb (K,d)):
    dist(x, k)    = ||x||^2 + ||cb_k||^2 - 2 x.cb_k      (fp32)
    idx           = argmin_k dist
    prob          = softmax(-dist)
    z_q           = cb[idx]  (+ straight-through: z + (z_q - z))
    q_loss        = 1.25 * mean((z_q - z)^2)

Precision strategy: the reference's argmin is decided by fp32 rounding of
dist at magnitude ~512 (ulp 6.1e-5) while top-2 gaps go down to 1e-8, so we
mirror the reference's fp32 arithmetic exactly:
    t1 = fp32(A + B)  with A=||x||^2 (host, numpy pairwise sum = XLA CPU),
                           B=||cb||^2 broadcast row
    C2 = 2*x.cb accumulated in PSUM via a 3-pass bf16 hi/lo split (the
         factor 2 is folded into the codebook operand; exact under fp32)
    nd = fp32(C2 - t1) = -fp32(t1 - C2) = -dist  elementwise
    argmax(nd) == argmin(dist), first-occurrence tie-break on both sides.
Validated: 0/32768 argmin mismatches vs the jax-CPU reference on HW.

Per core (batch b): z[b] is natively (d=512, tok=4096), exactly the
transposed stationary operand the TensorEngine wants. The z_q gather is an
indirect DMA over codebook rows; its (tok, d) -> (d, tok) transpose runs on
the TensorEngine one tile behind the matmul stream so PE never stalls on
the argmax -> gather chain.
"""
import sys

sys.path.insert(0, "/opt/trn_rl_repo")

import time

import numpy as np
import ml_dtypes

import concourse.bacc as bacc
import concourse.bass as bass
import concourse.mybir as mybir
import concourse.tile as tile
from concourse.bass_utils import run_bass_kernel_spmd
from concourse.masks import make_identity

B, D, H, W, K = 8, 512, 64, 64, 1024
N = H * W          # tokens per core
NT = N // 128      # 32 token tiles per core
DC = D // 128      # 4 contraction chunks
BETA = 0.25

f32 = mybir.dt.float32
bf16 = mybir.dt.bfloat16
u32 = mybir.dt.uint32

_CACHE = {}


def _build():
    nc = bacc.Bacc()

    z = nc.dram_tensor("z", [D, N], f32, kind="ExternalInput")
    zp = nc.dram_tensor("zp", [D, N], f32, kind="ExternalInput")
    cbh2 = nc.dram_tensor("cbh2", [D, K], bf16, kind="ExternalInput")
    cbl2 = nc.dram_tensor("cbl2", [D, K], bf16, kind="ExternalInput")
    cbtab = nc.dram_tensor("cbtab", [K, D], f32, kind="ExternalInput")
    arow = nc.dram_tensor("arow", [128, NT], f32, kind="ExternalInput")
    brow = nc.dram_tensor("brow", [1, K], f32, kind="ExternalInput")
    nbrow = nc.dram_tensor("nbrow", [1, K], bf16, kind="ExternalInput")

    prob = nc.dram_tensor("prob", [N, K], f32, kind="ExternalOutput")
    pprob = nc.dram_tensor("pprob", [N, K], f32, kind="ExternalOutput")
    zq = nc.dram_tensor("zq", [D, N], f32, kind="ExternalOutput")
    idx = nc.dram_tensor("idx", [128, NT], u32, kind="ExternalOutput")
    lossp = nc.dram_tensor("lossp", [128, 1], f32, kind="ExternalOutput")

    z_r = z.rearrange("(c p) n -> p c n", p=128)
    zp_r = zp.rearrange("(c p) n -> p c n", p=128)
    zq_r = zq.rearrange("(c p) n -> p c n", p=128)
    Exp = mybir.ActivationFunctionType.Exp
    Square = mybir.ActivationFunctionType.Square
    Copy = mybir.ActivationFunctionType.Copy

    with tile.TileContext(nc) as tc:
        with (
            tc.tile_pool(name="const", bufs=1) as cpool,
            tc.tile_pool(name="sbuf", bufs=3) as sp,
            tc.tile_pool(name="psum", bufs=1, space="PSUM") as pp,
        ):
            # ---------------- constants ----------------
            cbh2_r = cbh2.rearrange("(c p) k -> p c k", p=128)
            cbl2_r = cbl2.rearrange("(c p) k -> p c k", p=128)
            ch = cpool.tile([128, DC, K], bf16, tag="cbh2")
            cl = cpool.tile([128, DC, K], bf16, tag="cbl2")
            for ki in range(2):
                ks = slice(ki * 512, (ki + 1) * 512)
                nc.sync.dma_start(out=ch[:, :, ks], in_=cbh2_r[:, :, ks])
                nc.sync.dma_start(out=cl[:, :, ks], in_=cbl2_r[:, :, ks])
            at = cpool.tile([128, NT], f32, tag="arow")
            nc.sync.dma_start(out=at[:], in_=arow[:])
            bt = cpool.tile([1, K], f32, tag="brow")
            nc.sync.dma_start(out=bt[:], in_=brow[:])
            nbt = cpool.tile([1, K], bf16, tag="nbrow")
            nc.sync.dma_start(out=nbt[:], in_=nbrow[:])
            ones = cpool.tile([1, 128], f32, tag="ones")
            nc.vector.memset(ones[:], 1.0)
            onesb = cpool.tile([1, 128], bf16, tag="onesb")
            nc.vector.memset(onesb[:], 1.0)
            ident = cpool.tile([128, 128], f32, tag="ident")
            make_identity(nc, ident[:])
            lossacc = cpool.tile([128, 1], f32, tag="lossacc")
            nc.vector.memset(lossacc[:], 0.0)
            idxacc = cpool.tile([128, NT], u32, tag="idxacc")

            # replicate B across partitions: ones(1,128).T @ B(1,K)
            bbc = cpool.tile([128, K], f32, tag="bbc")
            for ki in range(2):
                ks = slice(ki * 512, (ki + 1) * 512)
                bb_ps = pp.tile([128, 512], f32, tag="nd", bufs=3)
                nc.tensor.matmul(bb_ps[:], ones[:], bt[:, ks], start=True, stop=True)
                nc.vector.tensor_copy(bbc[:, ks], bb_ps[:])

            # software pipeline state: gather tile + z tile of previous iter
            prev = None

            def emit_tail(pv):
                """Transpose prev tile's gathered rows and produce zq/loss."""
                gt, zt_p, tsl = pv
                zqg = sp.tile([128, DC, 128], f32, tag="zqg")
                for c in range(DC):
                    tr_ps = pp.tile([128, 128], f32, tag="tr", bufs=2)
                    nc.tensor.transpose(
                        out=tr_ps[:], in_=gt[:, c * 128 : (c + 1) * 128],
                        identity=ident[:],
                    )
                    nc.scalar.copy(zqg[:, c, :], tr_ps[:])
                # diff = z_q - z ; zq_out = z + diff  (mirrors reference STE)
                diff = sp.tile([128, DC, 128], f32, tag="diff")
                nc.vector.tensor_sub(diff[:], zqg[:], zt_p[:])
                zqt = sp.tile([128, DC, 128], f32, tag="zqt")
                nc.gpsimd.tensor_add(zqt[:], diff[:], zt_p[:])
                nc.sync.dma_start(out=zq_r[:, :, tsl], in_=zqt[:])
                sq = sp.tile([128, DC, 128], f32, tag="sq")
                lp = sp.tile([128, 1], f32, tag="lp")
                nc.scalar.activation(sq[:], diff[:], Square, accum_out=lp[:])
                nc.vector.tensor_add(lossacc[:], lossacc[:], lp[:])

            for t in range(NT):
                ts = slice(t * 128, (t + 1) * 128)
                zt = sp.tile([128, DC, 128], f32, tag="zt", bufs=6)
                nc.sync.dma_start(out=zt[:], in_=z_r[:, :, ts])
                zh = sp.tile([128, DC, 128], bf16, tag="zh", bufs=4)
                nc.scalar.copy(zh[:], zt[:])
                zl = sp.tile([128, DC, 128], bf16, tag="zl", bufs=4)
                nc.vector.tensor_sub(zl[:], zt[:], zh[:])
                zpt = sp.tile([128, DC, 128], f32, tag="zpt")
                nc.sync.dma_start(out=zpt[:], in_=zp_r[:, :, ts])
                zph = sp.tile([128, DC, 128], bf16, tag="zph", bufs=4)
                nc.scalar.copy(zph[:], zpt[:])

                # ---- PE: z 3-pass (24 MM) ----
                nd_ps = pp.tile([128, K], f32, tag="nd", bufs=3)
                for ki in range(2):
                    ks = slice(ki * 512, (ki + 1) * 512)
                    first = True
                    for zi, ci in ((zh, ch), (zl, ch), (zh, cl)):
                        for c in range(DC):
                            nc.tensor.matmul(
                                nd_ps[:, ks],
                                zi[:, c, :],
                                ci[:, c, ks],
                                start=first,
                                stop=(zi is zh and ci is cl and c == DC - 1),
                            )
                            first = False

                # ---- PE: z_pos 1-pass + B fold (10 MM) ----
                ndp_ps = pp.tile([128, K], f32, tag="nd", bufs=3)
                for ki in range(2):
                    ks = slice(ki * 512, (ki + 1) * 512)
                    for c in range(DC):
                        nc.tensor.matmul(
                            ndp_ps[:, ks],
                            zph[:, c, :],
                            ch[:, c, ks],
                            start=(c == 0),
                            stop=False,
                        )
                    nc.tensor.matmul(
                        ndp_ps[:, ks], onesb[:], nbt[:, ks], start=False, stop=True
                    )

                # ---- PE: transposes for previous tile (no stall: gather done) --
                if prev is not None:
                    emit_tail(prev)

                # ---- z: mirror dist, argmax, softmax ----
                t1 = sp.tile([128, K], f32, tag="t1")
                nc.vector.tensor_scalar(
                    t1[:], bbc[:], at[:, t : t + 1], None, op0=mybir.AluOpType.add
                )
                ndt = sp.tile([128, K], f32, tag="ndt")
                nc.vector.tensor_sub(ndt[:], nd_ps[:], t1[:])

                mx = sp.tile([128, 8], f32, tag="mx")
                nc.vector.max(out=mx[:], in_=ndt[:])
                mi = sp.tile([128, 8], u32, tag="mi")
                nc.vector.max_index(out=mi[:], in_max=mx[:], in_values=ndt[:])
                nc.vector.tensor_copy(idxacc[:, t : t + 1], mi[:, 0:1])
                negmax = sp.tile([128, 1], f32, tag="negmax")
                nc.scalar.activation(negmax[:], mx[:, 0:1], Copy, scale=-1.0)

                et = sp.tile([128, K], f32, tag="et")
                sume = sp.tile([128, 1], f32, tag="sume")
                nc.scalar.activation(
                    et[:], ndt[:], Exp, bias=negmax[:, 0:1], accum_out=sume[:]
                )
                rcp = sp.tile([128, 1], f32, tag="rcp")
                nc.vector.reciprocal(rcp[:], sume[:])
                pt = sp.tile([128, K], f32, tag="pt")
                nc.vector.tensor_scalar_mul(pt[:], et[:], rcp[:, 0:1])
                nc.sync.dma_start(out=prob[ts, :], in_=pt[:])

                # ---- z_pos: softmax straight from PSUM ----
                ep = sp.tile([128, K], f32, tag="et")
                sump = sp.tile([128, 1], f32, tag="sume")
                nc.scalar.activation(ep[:], ndp_ps[:], Exp, accum_out=sump[:])
                rcpp = sp.tile([128, 1], f32, tag="rcp")
                nc.vector.reciprocal(rcpp[:], sump[:])
                ppt = sp.tile([128, K], f32, tag="pt")
                nc.gpsimd.tensor_scalar_mul(ppt[:], ep[:], rcpp[:, 0:1])
                nc.sync.dma_start(out=pprob[ts, :], in_=ppt[:])

                # ---- gather z_q rows for this tile (transposed next iter) ----
                gt = sp.tile([128, D], f32, tag="gt")
                nc.gpsimd.indirect_dma_start(
                    out=gt[:],
                    out_offset=None,
                    in_=cbtab[:],
                    in_offset=bass.IndirectOffsetOnAxis(ap=mi[:, 0:1], axis=0),
                )
                prev = (gt, zt, ts)

            emit_tail(prev)

            nc.sync.dma_start(out=idx[:], in_=idxacc[:])
            nc.sync.dma_start(out=lossp[:], in_=lossacc[:])

    nc.finalize()
    return nc


def _get_nc():
    if "nc" not in _CACHE:
        _CACHE["nc"] = _build()
    return _CACHE["nc"]


def kernel(z, z_pos, codebook):
    z = np.ascontiguousarray(np.asarray(z, dtype=np.float32))
    z_pos = np.ascontiguousarray(np.asarray(z_pos, dtype=np.float32))
    cb = np.ascontiguousarray(np.asarray(codebook, dtype=np.float32))

    # host prep: codebook derivatives (tiny) + per-token ||x||^2 (mirrors
    # the reference's fp32 row-sum via numpy pairwise summation)
    cbT2 = np.ascontiguousarray(cb.T) * np.float32(2.0)        # (D, K)
    cbh2 = cbT2.astype(ml_dtypes.bfloat16)
    cbl2 = (cbT2 - cbh2.astype(np.float32)).astype(ml_dtypes.bfloat16)
    brow = (cb * cb).sum(axis=1).astype(np.float32)[None, :]   # (1, K)
    nbrow = (-brow).astype(ml_dtypes.bfloat16)

    zf = z.transpose(0, 2, 3, 1).reshape(-1, D)
    A = (zf * zf).sum(axis=1)                                  # fp32 pairwise

    in_maps = []
    for b in range(B):
        arow_b = np.ascontiguousarray(
            A[b * N : (b + 1) * N].reshape(NT, 128).T
        )  # (128, NT): arow[p, t] = A[t*128 + p]
        in_maps.append(
            {
                "z": z[b].reshape(D, N),
                "zp": z_pos[b].reshape(D, N),
                "cbh2": cbh2,
                "cbl2": cbl2,
                "cbtab": cb,
                "arow": arow_b,
                "brow": brow,
                "nbrow": nbrow,
            }
        )

    nc = _get_nc()
    t0 = time.perf_counter()
    res = run_bass_kernel_spmd(nc, in_maps, core_ids=list(range(B)))
    t1 = time.perf_counter()
    _CACHE["last_exec_wall_s"] = t1 - t0

    z_q = np.empty((B, D, H, W), dtype=np.float32)
    dist_prob = np.empty((B, H, W, K), dtype=np.float32)
    pos_prob = np.empty((B, H, W, K), dtype=np.float32)
    indices = np.empty((B, H, W), dtype=np.int32)
    loss_sum = 0.0
    for b in range(B):
        r = res.results[b]
        z_q[b] = r["zq"].reshape(D, H, W)
        dist_prob[b] = r["prob"].reshape(H, W, K)
        pos_prob[b] = r["pprob"].reshape(H, W, K)
        indices[b] = r["idx"].T.reshape(H, W).astype(np.int32)
        loss_sum += r["lossp"].astype(np.float64).sum()

    q_loss = np.float32((1.0 + BETA) * loss_sum / (B * N * D))
    return z_q, q_loss, dist_prob, indices, pos_prob
